# revision 10
# baseline (speedup 1.0000x reference)
"""Trainium2 Bass kernel for a channel-attention block.

Per batch b (one NeuronCore each, 8 total):
    v      = x[b].reshape(C, H*W)                    # [256, 16384]
    energy = v @ v.T                                 # [256, 256]
    w      = softmax(max(energy, -1) - energy, -1)   # == softmax(-energy)
    y      = alpha * (w @ v) + x[b]

Layout / strategy (per core), v2 (fp16 compute path):
  - x is cast f32->fp16 during the input DMA (SWDGE) into a resident
    v_sb [128, 2, 16384] fp16 (c = h*128 + p).  fp16 keeps the energy
    logit noise ~30x below bf16 (rel-l2 ~2e-3 vs ~1.7e-2, gate 2e-2).
  - Phase B: each 128-wide s-tile is PE-transposed (fp16: 1 cycle/row
    vs f32r's 1.5) and fed to two fp16 matmuls accumulating [128, 256]
    PSUM tiles.  PE per-tile work (320 ns) now beats the DMA stream
    (366 ns/tile), so k-tile 0's DMA is deferred behind tiles 1-12:
    the PE starts ~7 us into the stream and runs gap-free at full
    p-state, finishing ~0.5 us after the last tile lands.
  - Phase C: stable softmax via reduce-min + fused exp(min - e) with
    accumulated row-sum.  The 1/sum (and alpha) are NOT applied to w;
    they fold into phase D's scalar_tensor_tensor as a per-partition
    scalar.  w and its PE-transpose wT stay fp16.
  - Phase D: y = rc*alpha*(wHat @ v) + x fused on VectorE reading PSUM
    + fp16 v; output staged in SBUF and streamed with 512 KB head/tail
    pieces (DMA starts ~2 us after softmax) and 2 MB steady chunks.
"""

from contextlib import ExitStack

import numpy as np

import concourse.bass as bass
import concourse.mybir as mybir
import concourse.tile as tile
from concourse import bacc
from concourse.bass_utils import run_bass_kernel_spmd
from concourse.masks import make_identity

B, C, HH, WW = 8, 256, 128, 128
HW = HH * WW            # 16384
P = 128
H = C // P              # 2 channel chunks
KT = HW // P            # 128 contraction tiles for energy
S_TILE = 512            # second-matmul moving free dim (1 PSUM bank)

F32 = mybir.dt.float32
F16 = mybir.dt.float16

# Input DMA pieces in k-tiles.  Tile 0 deferred behind 1-8: the PE's
# first transpose then fires at ~5.4 us, late enough that the (faster)
# PE never outruns the stream and runs gap-free at full p-state to the
# last tile.
IN_PIECES = [(1, 5), (5, 9), (0, 1), (9, 13)] + [
    (t, min(t + 16, KT)) for t in range(13, KT, 16)
]
# Output DMA piece widths in columns: 512 KB head x4 (early DMA start),
# graded up to 2 MB steady, 512 KB tail x4 (small kernel-gating final
# DMA).  Grading keeps each piece's staging barrier under its DMA time.
OUT_PIECES = [512] * 4 + [1024] * 2 + [2048] * 5 + [512] * 4
assert sum(OUT_PIECES) == HW


def emit(nc, tc, alpha, ident_h, v_sb, x_v, y_v):
    """One full per-core pass (phases A-D). Pools are scoped inside."""
    # ---- Phase A: stream x into SBUF as fp16 (SWDGE cast DMA).
    for t0, t1 in IN_PIECES:
        sl = slice(t0 * P, t1 * P)
        nc.gpsimd.dma_start(out=v_sb[:, :, sl], in_=x_v[:, :, sl])

    with ExitStack() as wctx:
        w_pool = wctx.enter_context(tc.tile_pool(name="w", bufs=1))
        w_sb = [w_pool.tile([P, C], F16, name=f"w{h}") for h in range(H)]
        wt_sb = [w_pool.tile([P, C], F16, name=f"wt{g}") for g in range(H)]
        stats = wctx.enter_context(tc.tile_pool(name="stats", bufs=1))
        rca = [stats.tile([P, 1], F32, name=f"rca{h}") for h in range(H)]

        with ExitStack() as bctx:
            vt_pool = bctx.enter_context(tc.tile_pool(name="vt", bufs=8))
            psum_e = bctx.enter_context(
                tc.tile_pool(name="psum_e", bufs=1, space="PSUM"))
            psum_t = bctx.enter_context(
                tc.tile_pool(name="psum_t", bufs=6, space="PSUM"))

            # ---- Phase B: energy = v @ v.T (two PSUM banks), with the
            # transpose+copy pipelined ahead of the matmuls.  PSUM tiles are
            # padded to a full 2 KB bank each so transposes, copies, and the
            # e_ps accumulators never share a bank (write/read hazards
            # serialize at bank granularity).
            e_ps = [psum_e.tile([P, 2 * C], F32, name=f"energy{h}")[:, 0:C]
                    for h in range(H)]

            def make_vt(k):
                ksl = slice(k * P, (k + 1) * P)
                vt = vt_pool.tile([P, C], F16, name="vt")
                tp = psum_t.tile([P, 4, C], F16, name="tp")[:, 0, :]
                for h in range(H):
                    nc.tensor.transpose(
                        tp[:, h * P:(h + 1) * P], v_sb[:, h, ksl], ident_h[:]
                    )
                if k % 2 == 0:
                    nc.scalar.copy(vt[:], tp)
                else:
                    nc.vector.tensor_copy(vt[:], tp)
                return vt

            # Two k-tiles per pipeline step; transposes+copies run three
            # pairs ahead of the matmuls so the copy latency chain never
            # stalls the PE.
            vts = [make_vt(0), make_vt(1), make_vt(2), make_vt(3)]
            for k0 in range(0, KT, 2):
                for kn in (k0 + 4, k0 + 5):
                    if kn < KT:
                        vts.append(make_vt(kn))
                for k in (k0, k0 + 1):
                    vt_r = vts.pop(0)[:]
                    for h in range(H):
                        nc.tensor.matmul(
                            e_ps[h],
                            lhsT=vt_r[:, h * P:(h + 1) * P],
                            rhs=vt_r,
                            start=(k == 0),
                            stop=(k == KT - 1),
                        )

            # ---- Phase C: softmax(max - e) == exp(min - e)/sum.  Only the
            # exp is materialized (fp16); 1/sum * alpha folds into phase D.
            # The two row-min reduces run on different engines in parallel.
            for h in range(H):
                mn = stats.tile([P, 1], F32, name=f"mn{h}")
                sm = stats.tile([P, 1], F32, name=f"sm{h}")
                rc = stats.tile([P, 1], F32, name=f"rc{h}")
                nc.vector.tensor_reduce(
                    mn[:], e_ps[h], axis=mybir.AxisListType.X,
                    op=mybir.AluOpType.min
                )
                nc.scalar.activation(
                    w_sb[h][:], e_ps[h], mybir.ActivationFunctionType.Exp,
                    bias=mn[:], scale=-1.0, accum_out=sm[:],
                )
                nc.vector.reciprocal(rc[:], sm[:])
                nc.vector.tensor_scalar_mul(rca[h][:], rc[:], float(alpha))
            # wT[g][p, h*128+q] = w[h][q, g*128+p] for the second matmul.
            for g in range(H):
                for h in range(H):
                    tp2 = psum_t.tile([P, 4, C], F16, name="tp2", tag="tp")
                    nc.tensor.transpose(
                        tp2[:, 0, 0:P], w_sb[h][:, g * P:(g + 1) * P],
                        ident_h[:]
                    )
                    # h=1 is softmax-critical-path-last: keep its copies on
                    # the faster DVE; h=0's go to ACT.
                    if h == 1:
                        nc.vector.tensor_copy(
                            wt_sb[g][:, h * P:(h + 1) * P], tp2[:, 0, 0:P])
                    else:
                        nc.scalar.copy(
                            wt_sb[g][:, h * P:(h + 1) * P], tp2[:, 0, 0:P])

        # ---- Phase D: y = rc*alpha*(wHat @ v) + v, streamed out.
        with ExitStack() as dctx:
            out_s = dctx.enter_context(tc.tile_pool(name="out_s", bufs=3))
            out_b = dctx.enter_context(tc.tile_pool(name="out_b", bufs=3))
            psum_y = dctx.enter_context(
                tc.tile_pool(name="psum_y", bufs=4, space="PSUM"))
            col = 0
            for width in OUT_PIECES:
                pool = out_s if width == S_TILE else out_b
                ost = pool.tile([P, H, width], F32, name="ost")
                for m in range(H):
                    for jj in range(width // S_TILE):
                        j0 = col + jj * S_TILE
                        jsl = slice(j0, j0 + S_TILE)
                        yp = psum_y.tile([P, S_TILE], F32, name="yp")
                        for g in range(H):
                            nc.tensor.matmul(
                                yp[:],
                                lhsT=wt_sb[g][:][:, m * P:(m + 1) * P],
                                rhs=v_sb[:][:, g, jsl],
                                start=(g == 0),
                                stop=(g == H - 1),
                            )
                        # Single staging lane: GPSIMD cannot read PSUM on
                        # TRN2 (BIR verifier), so all staging runs on DVE.
                        nc.vector.scalar_tensor_tensor(
                            out=ost[:, m, jj * S_TILE:(jj + 1) * S_TILE],
                            in0=yp[:],
                            scalar=rca[m][:],
                            in1=v_sb[:, m, jsl],
                            op0=mybir.AluOpType.mult,
                            op1=mybir.AluOpType.add,
                        )
                osl = slice(col, col + width)
                nc.sync.dma_start(out=y_v[:, :, osl], in_=ost[:])
                col += width


def _build(alpha: float) -> bass.Bass:
    # Bacc (not plain Bass): its compile() legalizes semaphore waits into
    # EventSemaphore instructions — hardware allows only 1 wait per
    # instruction and Tile freely emits more.
    nc = bacc.Bacc("TRN2", target_bir_lowering=False)
    x = nc.dram_tensor("x", [C, HW], F32, kind="ExternalInput")
    y = nc.dram_tensor("y", [C, HW], F32, kind="ExternalOutput")
    x_v = x.rearrange("(h p) s -> p h s", p=P)
    y_v = y.rearrange("(h p) s -> p h s", p=P)

    with tile.TileContext(nc) as tc, ExitStack() as ctx:
        singles = ctx.enter_context(tc.tile_pool(name="singles", bufs=1))
        ident = singles.tile([P, P], F32, name="ident")
        make_identity(nc, ident)
        ident_h = singles.tile([P, P], F16, name="ident_h")
        nc.vector.tensor_copy(ident_h[:], ident[:])
        # Whole v resident as fp16: 64 KB per partition.
        v_sb = singles.tile([P, H, HW], F16, name="v_sb")
        emit(nc, tc, alpha, ident_h, v_sb, x_v, y_v)
    nc.compile()
    return nc


def kernel(x: np.ndarray, alpha: np.ndarray, **_kw) -> np.ndarray:
    assert x.shape == (B, C, HH, WW) and x.dtype == np.float32
    xs = np.ascontiguousarray(x.reshape(B, C, HW)).astype(np.float32, copy=False)
    nc = _build(float(np.asarray(alpha).reshape(-1)[0]))
    in_maps = [{"x": xs[b]} for b in range(B)]
    res = run_bass_kernel_spmd(nc, in_maps, core_ids=list(range(B)))
    out = np.stack([np.asarray(r["y"]) for r in res.results])
    return out.reshape(B, C, HH, WW).astype(np.float32, copy=False)
